# revision 14
# baseline (speedup 1.0000x reference)
"""Trainium2 Bass kernel for nn_Attention_64819646431478.

Single-layer causal attention, B=1, T=2048, DIM=1024, 16 heads, head_dim=64,
f32, with RMSNorm (eps=f32 eps) on Q and K heads.

Sharding: tensor-parallel over heads across 8 NeuronCores (2 heads/core).
Each core computes its heads' Q/K/V projections, causal attention, and the
partial output projection against its 128-row slice of w_o; the host sums
the 8 partial outputs (the "all-reduce" of the hint, done at gather time).

v2 layout/schedule notes:
  - All dram inputs are host-preswizzled so every DMA is a dense
    [128, n] copy (fast descriptor generation, early start); xT arrives
    in 32 (chunk-major) pieces so the first projection can begin after
    ~1MB instead of the full 4MB.
  - Prologue computes Q and K projections + RMS norms for all chunks
    using the Square/Sqrt ACT table set, then the main loop switches to
    the Exp set exactly once (table thrash costs 1.3us per switch).
  - Per-head 1/rms and 1/sum_exp broadcasts along partitions are done by
    gpsimd.partition_broadcast instead of PE matmuls; the normalize
    multiplies are fused scalar_tensor_tensor ops that also apply gamma.
  - Scores are computed transposed per 128-key tile: ST[tk, tq] = K@Q^T,
    exp on ACT per [128,512] tile with double-buffered PSUM so score
    matmuls never wait on exp; causal masks (bf16 multiply) only touch
    the 4 diagonal tiles per chunk.
  - The softmax denominator is free: V tiles carry a shared ones column
    (layout [V0 | 1 | V1]) so head0's PV matmul yields sums in row 64
    and head1's in row 0.
  - Output projection is emitted per (mu, chunk) with bf16 staging; the
    PSUM->SBUF copies alternate gpsimd/vector to keep DVE off the
    critical path, and the partial output is written in bf16 (host sums
    the 8 partials in f64).
"""

import os
import sys
import types

import numpy as np

# --- environment bootstrap (harness may run us from a bare directory) ---
for _p in ("/root/.axon_site", "/root/.axon_site/_ro/trn_rl_repo",
           "/root/.axon_site/_ro/pypackages", "/opt/trn_rl_repo"):
    if os.path.isdir(_p) and _p not in sys.path:
        sys.path.append(_p)


def _install_ntff_shim():
    """Provide antenv.axon_hooks (missing in this image) so trace=True works."""
    if "antenv.axon_hooks" in sys.modules:
        return
    mod = types.ModuleType("antenv.axon_hooks")
    mod._hook = None
    mod.set_axon_ntff_profile_hook = lambda h: setattr(mod, "_hook", h)
    mod.get_axon_ntff_profile_hook = lambda: mod._hook
    sys.modules["antenv.axon_hooks"] = mod
    try:
        import antenv
        antenv.axon_hooks = mod
        from trn_agent_boot.trn_boot import _ntff_profile_via_ctypes
        mod.set_axon_ntff_profile_hook(
            _ntff_profile_via_ctypes("/opt/axon/libaxon_pjrt.so"))
    except Exception:
        pass


_install_ntff_shim()

import ml_dtypes  # noqa: E402

import concourse.mybir as mybir  # noqa: E402
import concourse.tile as tile  # noqa: E402
from concourse import bacc  # noqa: E402

F32 = mybir.dt.float32
BF16 = mybir.dt.bfloat16
NP_BF16 = ml_dtypes.bfloat16
AF = mybir.ActivationFunctionType

T = 2048
C = 1024
D = 64
NCORES = 8
HPC = 2            # heads per core
JPC = HPC * D      # 128 j-columns per core
NTQ = 4            # tq chunks of 512
TQ = 512
NTK = 16           # tk tiles of 128
EPS = float(np.finfo(np.float32).eps)


def build_nc():
    nc = bacc.Bacc("TRN2", target_bir_lowering=False, debug=False,
                   num_devices=NCORES)

    xT_d = nc.dram_tensor("xT", [128, 8, T], BF16, kind="ExternalInput")
    wq_d = nc.dram_tensor("wq", [128, 8, 128], BF16, kind="ExternalInput")
    wk_d = nc.dram_tensor("wk", [128, 8, 128], BF16, kind="ExternalInput")
    wv_d = nc.dram_tensor("wv", [128, 8, 128], BF16, kind="ExternalInput")
    wo_d = nc.dram_tensor("wo", [128, C], BF16, kind="ExternalInput")
    masks_d = nc.dram_tensor("masks", [128, 4, TQ], BF16,
                             kind="ExternalInput")
    gq_d = nc.dram_tensor("gq", [128, 1], F32, kind="ExternalInput")
    gk_d = nc.dram_tensor("gk", [128, 1], F32, kind="ExternalInput")
    onescol_d = nc.dram_tensor("onescol", [128, 65], BF16,
                               kind="ExternalInput")
    oneh_d = nc.dram_tensor("oneh", [1, 256], BF16, kind="ExternalInput")
    ident_d = nc.dram_tensor("ident", [128, 128], BF16, kind="ExternalInput")
    outT_d = nc.dram_tensor("outT", [128, 8, T], BF16, kind="ExternalOutput")

    MUL = mybir.AluOpType.mult

    with tile.TileContext(nc) as tc, nc.allow_low_precision("bf16 kernel"):
        from contextlib import ExitStack
        with ExitStack() as ctx:
            consts = ctx.enter_context(tc.tile_pool(name="consts", bufs=1))
            acts = ctx.enter_context(tc.tile_pool(name="acts", bufs=1))
            ps_mm = ctx.enter_context(
                tc.tile_pool(name="ps_mm", bufs=3, space="PSUM"))

            # ---- inputs to SBUF; xT chunk-major on sync, weights on gpsimd
            wq_sb = consts.tile([128, 8, 128], BF16)
            wk_sb = consts.tile([128, 8, 128], BF16)
            wv_sb = consts.tile([128, 8, 128], BF16)
            wo_sb = consts.tile([128, C], BF16)
            msb = consts.tile([128, 4, TQ], BF16)
            gq_sb = consts.tile([128, 1], F32)
            gk_sb = consts.tile([128, 1], F32)
            ones2c = consts.tile([128, 65], BF16)
            oneh_sb = consts.tile([1, 256], BF16)
            ident_sb = consts.tile([128, 128], BF16)
            eps_sb = consts.tile([65, 1], F32)
            xT_sb = acts.tile([128, 8, T], BF16)

            nc.gpsimd.dma_start(out=wq_sb[:], in_=wq_d[:])
            nc.gpsimd.dma_start(out=wk_sb[:], in_=wk_d[:])
            for c4 in range(NTQ):
                sl = slice(TQ * c4, TQ * (c4 + 1))
                for ci in range(8):
                    nc.sync.dma_start(out=xT_sb[:, ci, sl],
                                      in_=xT_d[:, ci, sl])
            nc.gpsimd.dma_start(out=wv_sb[:], in_=wv_d[:])
            nc.gpsimd.dma_start(out=msb[:], in_=masks_d[:])
            nc.gpsimd.dma_start(out=wo_sb[:], in_=wo_d[:])
            nc.gpsimd.dma_start(out=gq_sb[:], in_=gq_d[:])
            nc.gpsimd.dma_start(out=gk_sb[:], in_=gk_d[:])
            nc.gpsimd.dma_start(out=ones2c[:], in_=onescol_d[:])
            nc.gpsimd.dma_start(out=oneh_sb[:], in_=oneh_d[:])
            nc.gpsimd.dma_start(out=ident_sb[:], in_=ident_d[:])
            nc.vector.memset(eps_sb[:], EPS)

            # ---- persistent activations ----
            QTn = acts.tile([128, T], BF16)     # [ (h,d), t ] normalized Q^T
            KTn = acts.tile([128, T], BF16)
            # V tiles: [V0(0:64) | ones(64) | V1(65:129) | ones(129)]
            # (per-head ones columns keep every PV output slice at an
            # aligned partition base)
            V_sb = acts.tile([128, NTK, 130], BF16)
            vview = V_sb[:].rearrange("p r (a b) -> p r a b", b=65)
            nc.vector.memset(vview[:, :, :, 64:65], 1.0)
            # rms chains (recip_fast needs base partition 0 per operand)
            # heads live at partitions {0, 64} so every 1-partition
            # access below has an aligned base (BIR requirement)
            rms_q = acts.tile([65, T], F32)
            rms_k = acts.tile([65, T], F32)
            rrf_q = acts.tile([65, T], F32)
            rrf_k = acts.tile([65, T], F32)
            # per-head 1/rms rows at base partition 0 (gpsimd bcast needs
            # an aligned base partition)
            rec_q = [acts.tile([1, T], BF16, name=f"rec_q{h}")
                     for h in range(HPC)]
            rec_k = [acts.tile([1, T], BF16, name=f"rec_k{h}")
                     for h in range(HPC)]
            sg = [acts.tile([1, T], F32, name=f"sg{h}") for h in range(HPC)]
            sgf = [acts.tile([1, T], F32, name=f"sgf{h}") for h in range(HPC)]
            sgr = [acts.tile([1, T], BF16, name=f"sgr{h}")
                   for h in range(HPC)]

            # ============ Prologue: Q/K projections + RMS norms ============
            with (
                tc.tile_pool(name="sqp", bufs=3) as sqp,
                tc.tile_pool(name="bbp", bufs=4) as bbp,
                tc.tile_pool(name="ps_sums", bufs=1, space="PSUM") as ps_sums,
                tc.tile_pool(name="ps_bb", bufs=2, space="PSUM") as ps_bb,
            ):
                for c4 in range(NTQ):
                    sl = slice(TQ * c4, TQ * (c4 + 1))
                    for w_sb, g_sb, rms_sb, rrf_sb, rec_sb, dst in (
                        (wq_sb, gq_sb, rms_q, rrf_q, rec_q, QTn),
                        (wk_sb, gk_sb, rms_k, rrf_k, rec_k, KTn),
                    ):
                        pp = ps_mm.tile([128, TQ], F32, tag="mm", name="pp")
                        for ci in range(8):
                            nc.tensor.matmul(
                                pp[:], w_sb[:, ci, :], xT_sb[:, ci, sl],
                                start=(ci == 0), stop=(ci == 7))
                        sq = sqp.tile([128, TQ], BF16, tag="sq", name="sq")
                        nc.scalar.activation(sq[:], pp[:], AF.Square)
                        # raw projection to SBUF (pp bank frees quickly)
                        raw = sqp.tile([128, TQ], BF16, tag="raw",
                                       name="raw")
                        nc.vector.tensor_copy(raw[:], pp[:])
                        sums = ps_sums.tile([65, TQ], F32, tag="sums",
                                            name="sums")
                        # ones-reduce over the 64 d-partitions per head:
                        # lhsT = sq (stationary) is invalid (M=512>128), so
                        # keep the onescol trick: [128,2] ones-blocks.
                        nc.tensor.matmul(sums[:], ones2c[:], sq[:],
                                         start=True, stop=True)
                        nc.scalar.activation(rms_sb[:, sl], sums[:],
                                             AF.Sqrt, bias=eps_sb[:],
                                             scale=1.0 / D)
                        nc.vector.reciprocal_approx_fast(
                            out=rrf_sb[:, sl], in_=rms_sb[:, sl])
                        # broadcast 1/rms along partitions via a K=1
                        # outer-product matmul (gpsimd partition ops cost
                        # ~13us on the Q7 cores - far too slow)
                        bb = ps_bb.tile([128, TQ], F32, tag="bb", name="bb")
                        for h in range(HPC):
                            nc.vector.tensor_copy(
                                rec_sb[h][0:1, sl],
                                rrf_sb[64 * h:64 * h + 1, sl])
                            nc.tensor.matmul(
                                bb[:], oneh_sb[0:1, 128 * h:128 * (h + 1)],
                                rec_sb[h][0:1, sl],
                                start=(h == 0), stop=(h == 1))
                        # dst = (raw * gamma[p]) * bb
                        nc.vector.scalar_tensor_tensor(
                            out=dst[:, sl], in0=raw[:], scalar=g_sb[:],
                            in1=bb[:], op0=MUL, op1=MUL)

            # ========== Main loop: V proj + attention, per chunk ==========
            with (
                tc.tile_pool(name="vtp", bufs=2) as vtp,
                tc.tile_pool(name="ep", bufs=6) as ep,
                tc.tile_pool(name="ctxp", bufs=2) as ctxp,
                tc.tile_pool(name="b2p", bufs=2) as b2p,
                tc.tile_pool(name="stgp", bufs=4) as stgp,
                tc.tile_pool(name="ps_tp", bufs=1, space="PSUM") as ps_tp,
                tc.tile_pool(name="ps_st", bufs=2, space="PSUM") as ps_st,
                tc.tile_pool(name="ps_ot", bufs=1, space="PSUM") as ps_ot,
            ):
                for c4 in range(NTQ):
                    sl = slice(TQ * c4, TQ * (c4 + 1))
                    n_tk = 4 * (c4 + 1)
                    # ---- V projection chunk + PE transpose into V_sb ----
                    pv = ps_mm.tile([128, TQ], F32, tag="mm", name="pv")
                    for ci in range(8):
                        nc.tensor.matmul(
                            pv[:], wv_sb[:, ci, :], xT_sb[:, ci, sl],
                            start=(ci == 0), stop=(ci == 7))
                    vt = vtp.tile([128, TQ], BF16, tag="vt", name="vt")
                    nc.vector.tensor_copy(vt[:], pv[:])
                    for rl in range(4):
                        r = 4 * c4 + rl
                        tp = ps_tp.tile([128, 128], BF16, tag="tp",
                                        name=f"tp{r}")
                        nc.tensor.transpose(tp[:],
                                            vt[:, 128 * rl:128 * (rl + 1)],
                                            ident_sb[:])
                        # single strided copy around the shared ones column
                        dst_v = V_sb[:, r, :].rearrange(
                            "p (a b) -> p a b", b=65)[:, :, 0:64]
                        src_v = tp[:].rearrange("p (a b) -> p a b", b=64)
                        nc.vector.tensor_copy(dst_v, src_v)

                    # ---- attention for this chunk ----
                    ot = [ps_ot.tile([65, TQ], F32, tag=f"ot{h}",
                                     name=f"ot{h}_{c4}")
                          for h in range(HPC)]
                    for r in range(n_tk):
                        diag = r - 4 * c4
                        for h in range(HPC):
                            hsl = slice(64 * h, 64 * (h + 1))
                            st = ps_st.tile([128, TQ], F32, tag="st",
                                            name=f"st{h}_{r}")
                            nc.tensor.matmul(
                                st[:], KTn[hsl, 128 * r:128 * (r + 1)],
                                QTn[hsl, sl], start=True, stop=True)
                            e_t = ep.tile([128, TQ], BF16, tag="e",
                                          name=f"e{h}_{r}")
                            nc.scalar.activation(e_t[:], st[:], AF.Exp,
                                                 scale=float(D) ** -0.5)
                            if diag >= 0:
                                nc.vector.tensor_mul(e_t[:], e_t[:],
                                                     msb[:, diag, :])
                            nc.tensor.matmul(
                                ot[h][:],
                                V_sb[:, r, 65 * h:65 * h + 65],
                                e_t[:],
                                start=(r == 0), stop=(r == n_tk - 1))
                    # ---- softmax denominators + normalize ----
                    ctxT = ctxp.tile([128, TQ], BF16, tag="ctx",
                                     name=f"ctx{c4}")
                    b2 = ps_mm.tile([128, TQ], F32, tag="mm",
                                    name=f"b2{c4}")
                    for h in range(HPC):
                        nc.vector.tensor_copy(sg[h][0:1, sl],
                                              ot[h][64:65, :])
                        nc.vector.reciprocal_approx_fast(
                            out=sgf[h][0:1, sl], in_=sg[h][0:1, sl])
                        nc.vector.tensor_copy(sgr[h][0:1, sl],
                                              sgf[h][0:1, sl])
                        nc.tensor.matmul(
                            b2[:], oneh_sb[0:1, 128 * h:128 * (h + 1)],
                            sgr[h][0:1, sl], start=(h == 0), stop=(h == 1))
                    # DVE cannot read two PSUM operands; stage b2 in SBUF
                    b2s = b2p.tile([128, TQ], BF16, tag="b2",
                                   name=f"b2s{c4}")
                    nc.vector.tensor_copy(b2s[:], b2[:])
                    for h in range(HPC):
                        nc.vector.scalar_tensor_tensor(
                            out=ctxT[64 * h:64 * (h + 1), :],
                            in0=ot[h][0:64, :],
                            scalar=1.0, in1=b2s[64 * h:64 * (h + 1), :],
                            op0=MUL, op1=MUL)
                    # ---- output projection, DMA per (mu, chunk) ----
                    for mu in range(8):
                        wop = ps_mm.tile([128, TQ], F32, tag="mm",
                                         name=f"wop{mu}_{c4}")
                        nc.tensor.matmul(wop[:],
                                         wo_sb[:, 128 * mu:128 * (mu + 1)],
                                         ctxT[:], start=True, stop=True)
                        stg = stgp.tile([128, TQ], BF16, tag="stg",
                                        name=f"stg{mu}_{c4}")
                        if mu % 2 == 0:
                            nc.vector.tensor_copy(stg[:], wop[:])
                        else:
                            nc.scalar.activation(stg[:], wop[:], AF.Copy)
                        nc.sync.dma_start(out=outT_d[:, mu, sl],
                                          in_=stg[:])

    nc.compile()
    return nc


_NC_CACHE = None


def _get_nc():
    global _NC_CACHE
    if _NC_CACHE is None:
        _NC_CACHE = build_nc()
    return _NC_CACHE


def _make_in_maps(x, w_q, w_k, w_v, w_o, q_gamma, k_gamma):
    x = np.asarray(x, dtype=np.float32).reshape(T, C)
    # xT[p, c, t] = x[t, c*128+p]
    xT = np.ascontiguousarray(
        x.reshape(T, 8, 128).transpose(2, 1, 0)).astype(NP_BF16)

    p = np.arange(128)
    f = np.arange(TQ)
    masks = np.zeros((128, 4, TQ), dtype=NP_BF16)
    for s in range(4):
        masks[:, s, :] = (f[None, :] >= (p[:, None] + 128 * s)).astype(
            NP_BF16)

    gq = np.tile(np.asarray(q_gamma, np.float32), 2).reshape(128, 1)
    gk = np.tile(np.asarray(k_gamma, np.float32), 2).reshape(128, 1)
    onescol = np.zeros((128, 65), dtype=NP_BF16)
    onescol[0:64, 0] = 1
    onescol[64:128, 64] = 1
    oneh = np.zeros((1, 256), dtype=NP_BF16)
    oneh[0, 0:64] = 1          # head0 block in first 128 cols
    oneh[0, 192:256] = 1       # head1 block in second 128 cols
    ident = np.eye(128, dtype=NP_BF16)

    common = dict(xT=xT, masks=masks, gq=gq, gk=gk, onescol=onescol,
                  oneh=oneh, ident=ident)

    in_maps = []
    for i in range(NCORES):
        rows = slice(JPC * i, JPC * (i + 1))

        def wsw(w):
            # [p, c, j] = W[rows][j, c*128+p]
            W = np.asarray(w, np.float32)[rows]           # [128, C]
            return np.ascontiguousarray(
                W.reshape(128, 8, 128).transpose(2, 1, 0)).astype(NP_BF16)

        wo = np.asarray(w_o, np.float32)[:, rows].T        # [128, C]
        in_maps.append(dict(common, wq=wsw(w_q), wk=wsw(w_k), wv=wsw(w_v),
                            wo=np.ascontiguousarray(wo).astype(NP_BF16)))
    return in_maps


def _run(x, w_q, w_k, w_v, w_o, q_gamma, k_gamma, trace=False):
    import time

    from concourse.bass_utils import run_bass_kernel_spmd
    nc = _get_nc()
    in_maps = _make_in_maps(x, w_q, w_k, w_v, w_o, q_gamma, k_gamma)
    res = None
    for attempt in range(3):
        try:
            res = run_bass_kernel_spmd(nc, in_maps, list(range(NCORES)),
                                       trace=trace)
            break
        except Exception:
            # rare transient NRT_EXEC_UNIT_UNRECOVERABLE under axon; the
            # terminal resets the device on the next load
            if attempt == 2:
                raise
            time.sleep(3.0)
    acc = np.zeros((128, 8, T), dtype=np.float64)
    for r in res.results:
        acc += r["outT"].astype(np.float64)
    # out[t, m*128+p] = acc[p, m, t]
    out = acc.transpose(2, 1, 0).reshape(T, C).astype(np.float32)
    return out.reshape(1, T, C), res


def kernel(x, w_q, w_k, w_v, w_o, q_gamma, k_gamma):
    out, _ = _run(x, w_q, w_k, w_v, w_o, q_gamma, k_gamma, trace=False)
    return out


# revision 16
# speedup vs baseline: 1.0200x; 1.0200x over previous
"""Trainium2 Bass kernel for nn_Attention_64819646431478.

Single-layer causal attention, B=1, T=2048, DIM=1024, 16 heads, head_dim=64,
f32, with RMSNorm (eps=f32 eps) on Q and K heads.

Sharding: tensor-parallel over heads across 8 NeuronCores (2 heads/core).
Each core computes its heads' Q/K/V projections, causal attention, and the
partial output projection against its 128-row slice of w_o; the host sums
the 8 partial outputs (the "all-reduce" of the hint, done at gather time).

v2 layout/schedule notes:
  - All dram inputs are host-preswizzled so every DMA is a dense
    [128, n] copy (fast descriptor generation, early start); xT arrives
    in 32 (chunk-major) pieces so the first projection can begin after
    ~1MB instead of the full 4MB.
  - Prologue computes Q and K projections + RMS norms for all chunks
    using the Square/Sqrt ACT table set, then the main loop switches to
    the Exp set exactly once (table thrash costs 1.3us per switch).
  - Per-head 1/rms and 1/sum_exp broadcasts along partitions are done by
    gpsimd.partition_broadcast instead of PE matmuls; the normalize
    multiplies are fused scalar_tensor_tensor ops that also apply gamma.
  - Scores are computed transposed per 128-key tile: ST[tk, tq] = K@Q^T,
    exp on ACT per [128,512] tile with double-buffered PSUM so score
    matmuls never wait on exp; causal masks (bf16 multiply) only touch
    the 4 diagonal tiles per chunk.
  - The softmax denominator is free: V tiles carry a shared ones column
    (layout [V0 | 1 | V1]) so head0's PV matmul yields sums in row 64
    and head1's in row 0.
  - Output projection is emitted per (mu, chunk) with bf16 staging; the
    PSUM->SBUF copies alternate gpsimd/vector to keep DVE off the
    critical path, and the partial output is written in bf16 (host sums
    the 8 partials in f64).
"""

import os
import sys
import types

import numpy as np

# --- environment bootstrap (harness may run us from a bare directory) ---
for _p in ("/root/.axon_site", "/root/.axon_site/_ro/trn_rl_repo",
           "/root/.axon_site/_ro/pypackages", "/opt/trn_rl_repo"):
    if os.path.isdir(_p) and _p not in sys.path:
        sys.path.append(_p)


def _install_ntff_shim():
    """Provide antenv.axon_hooks (missing in this image) so trace=True works."""
    if "antenv.axon_hooks" in sys.modules:
        return
    mod = types.ModuleType("antenv.axon_hooks")
    mod._hook = None
    mod.set_axon_ntff_profile_hook = lambda h: setattr(mod, "_hook", h)
    mod.get_axon_ntff_profile_hook = lambda: mod._hook
    sys.modules["antenv.axon_hooks"] = mod
    try:
        import antenv
        antenv.axon_hooks = mod
        from trn_agent_boot.trn_boot import _ntff_profile_via_ctypes
        mod.set_axon_ntff_profile_hook(
            _ntff_profile_via_ctypes("/opt/axon/libaxon_pjrt.so"))
    except Exception:
        pass


_install_ntff_shim()

import ml_dtypes  # noqa: E402

import concourse.mybir as mybir  # noqa: E402
import concourse.tile as tile  # noqa: E402
from concourse import bacc  # noqa: E402

F32 = mybir.dt.float32
BF16 = mybir.dt.bfloat16
NP_BF16 = ml_dtypes.bfloat16
AF = mybir.ActivationFunctionType

T = 2048
C = 1024
D = 64
NCORES = 8
HPC = 2            # heads per core
JPC = HPC * D      # 128 j-columns per core
NTQ = 4            # tq chunks of 512
TQ = 512
NTK = 16           # tk tiles of 128
EPS = float(np.finfo(np.float32).eps)


def build_nc():
    nc = bacc.Bacc("TRN2", target_bir_lowering=False, debug=False,
                   num_devices=NCORES)

    xT_d = nc.dram_tensor("xT", [128, 8, T], BF16, kind="ExternalInput")
    wq_d = nc.dram_tensor("wq", [128, 8, 128], BF16, kind="ExternalInput")
    wk_d = nc.dram_tensor("wk", [128, 8, 128], BF16, kind="ExternalInput")
    wv_d = nc.dram_tensor("wv", [128, 8, 128], BF16, kind="ExternalInput")
    wo_d = nc.dram_tensor("wo", [128, C], BF16, kind="ExternalInput")
    masks_d = nc.dram_tensor("masks", [128, 4, TQ], BF16,
                             kind="ExternalInput")
    gq_d = nc.dram_tensor("gq", [128, 1], F32, kind="ExternalInput")
    gk_d = nc.dram_tensor("gk", [128, 1], F32, kind="ExternalInput")
    onescol_d = nc.dram_tensor("onescol", [128, 65], BF16,
                               kind="ExternalInput")
    oneh_d = nc.dram_tensor("oneh", [1, 256], BF16, kind="ExternalInput")
    ident_d = nc.dram_tensor("ident", [128, 128], BF16, kind="ExternalInput")
    outT_d = nc.dram_tensor("outT", [128, 8, T], BF16, kind="ExternalOutput")

    MUL = mybir.AluOpType.mult

    with tile.TileContext(nc) as tc, nc.allow_low_precision("bf16 kernel"):
        from contextlib import ExitStack
        with ExitStack() as ctx:
            consts = ctx.enter_context(tc.tile_pool(name="consts", bufs=1))
            acts = ctx.enter_context(tc.tile_pool(name="acts", bufs=1))
            ps_mm = ctx.enter_context(
                tc.tile_pool(name="ps_mm", bufs=3, space="PSUM"))

            # ---- inputs to SBUF; xT chunk-major on sync, weights on gpsimd
            wq_sb = consts.tile([128, 8, 128], BF16)
            wk_sb = consts.tile([128, 8, 128], BF16)
            wv_sb = consts.tile([128, 8, 128], BF16)
            wo_sb = consts.tile([128, C], BF16)
            msb = consts.tile([128, 4, TQ], BF16)
            gq_sb = consts.tile([128, 1], F32)
            gk_sb = consts.tile([128, 1], F32)
            ones2c = consts.tile([128, 65], BF16)
            oneh_sb = consts.tile([1, 256], BF16)
            ident_sb = consts.tile([128, 128], BF16)
            eps_sb = consts.tile([65, 1], F32)
            xT_sb = acts.tile([128, 8, T], BF16)

            nc.gpsimd.dma_start(out=wq_sb[:], in_=wq_d[:])
            nc.gpsimd.dma_start(out=wk_sb[:], in_=wk_d[:])
            for c4 in range(NTQ):
                sl = slice(TQ * c4, TQ * (c4 + 1))
                for ci in range(8):
                    nc.sync.dma_start(out=xT_sb[:, ci, sl],
                                      in_=xT_d[:, ci, sl])
            nc.gpsimd.dma_start(out=wv_sb[:], in_=wv_d[:])
            nc.gpsimd.dma_start(out=msb[:], in_=masks_d[:])
            nc.gpsimd.dma_start(out=wo_sb[:], in_=wo_d[:])
            nc.gpsimd.dma_start(out=gq_sb[:], in_=gq_d[:])
            nc.gpsimd.dma_start(out=gk_sb[:], in_=gk_d[:])
            nc.gpsimd.dma_start(out=ones2c[:], in_=onescol_d[:])
            nc.gpsimd.dma_start(out=oneh_sb[:], in_=oneh_d[:])
            nc.gpsimd.dma_start(out=ident_sb[:], in_=ident_d[:])
            nc.vector.memset(eps_sb[:], EPS)

            # ---- persistent activations ----
            QTn = acts.tile([128, T], BF16)     # [ (h,d), t ] normalized Q^T
            KTn = acts.tile([128, T], BF16)
            # V tiles: [V0(0:64) | ones(64) | V1(65:129) | ones(129)]
            # (per-head ones columns keep every PV output slice at an
            # aligned partition base)
            V_sb = acts.tile([128, NTK, 130], BF16)
            vview = V_sb[:].rearrange("p r (a b) -> p r a b", b=65)
            nc.vector.memset(vview[:, :, :, 64:65], 1.0)
            # rms chains (recip_fast needs base partition 0 per operand)
            # heads live at partitions {0, 64} so every 1-partition
            # access below has an aligned base (BIR requirement)
            rms_q = acts.tile([65, T], F32)
            rms_k = acts.tile([65, T], F32)
            rrf_q = acts.tile([65, T], F32)
            rrf_k = acts.tile([65, T], F32)
            # per-head 1/rms rows at base partition 0 (gpsimd bcast needs
            # an aligned base partition)
            rec_q = [acts.tile([1, T], BF16, name=f"rec_q{h}")
                     for h in range(HPC)]
            rec_k = [acts.tile([1, T], BF16, name=f"rec_k{h}")
                     for h in range(HPC)]
            sg = [acts.tile([1, T], F32, name=f"sg{h}") for h in range(HPC)]
            sgf = [acts.tile([1, T], F32, name=f"sgf{h}") for h in range(HPC)]
            sgr = [acts.tile([1, T], BF16, name=f"sgr{h}")
                   for h in range(HPC)]

            # ============ Prologue: Q/K projections + RMS norms ============
            # (also V-proj chunk 0, so the main loop can start attention
            # immediately; V-proj for chunk c+1 is interleaved into
            # attention(c) as PE gap-filler)
            vtp = ctx.enter_context(tc.tile_pool(name="vtp", bufs=2))

            def v_proj_ops(c4):
                """Yield thunks: 8 proj matmuls, then 4x(transpose+copy)."""
                sl = slice(TQ * c4, TQ * (c4 + 1))
                pv = ps_mm.tile([128, TQ], F32, tag="mm", name=f"pv{c4}")
                for ci in range(8):
                    yield lambda ci=ci: nc.tensor.matmul(
                        pv[:], wv_sb[:, ci, :], xT_sb[:, ci, sl],
                        start=(ci == 0), stop=(ci == 7))
                vt = vtp.tile([128, TQ], BF16, tag="vt", name=f"vt{c4}")
                yield lambda: nc.vector.tensor_copy(vt[:], pv[:])
                for rl in range(4):
                    r = 4 * c4 + rl
                    def tpcopy(r=r, rl=rl):
                        tp = ps_tp.tile([128, 128], BF16, tag="tp",
                                        name=f"tp{r}")
                        nc.tensor.transpose(
                            tp[:], vt[:, 128 * rl:128 * (rl + 1)],
                            ident_sb[:])
                        dst_v = V_sb[:, r, :].rearrange(
                            "p (a b) -> p a b", b=65)[:, :, 0:64]
                        src_v = tp[:].rearrange("p (a b) -> p a b", b=64)
                        nc.vector.tensor_copy(dst_v, src_v)
                    yield tpcopy

            with (
                tc.tile_pool(name="sqp", bufs=3) as sqp,
                tc.tile_pool(name="ps_sums", bufs=1, space="PSUM") as ps_sums,
                tc.tile_pool(name="ps_bb", bufs=2, space="PSUM") as ps_bb,
                tc.tile_pool(name="ps_tp", bufs=2, space="PSUM") as ps_tp,
            ):
                for c4 in range(NTQ):
                    sl = slice(TQ * c4, TQ * (c4 + 1))
                    for w_sb, g_sb, rms_sb, rrf_sb, rec_sb, dst in (
                        (wq_sb, gq_sb, rms_q, rrf_q, rec_q, QTn),
                        (wk_sb, gk_sb, rms_k, rrf_k, rec_k, KTn),
                    ):
                        pp = ps_mm.tile([128, TQ], F32, tag="mm", name="pp")
                        for ci in range(8):
                            nc.tensor.matmul(
                                pp[:], w_sb[:, ci, :], xT_sb[:, ci, sl],
                                start=(ci == 0), stop=(ci == 7))
                        sq = sqp.tile([128, TQ], BF16, tag="sq", name="sq")
                        nc.scalar.activation(sq[:], pp[:], AF.Square)
                        # raw projection to SBUF (pp bank frees quickly)
                        raw = sqp.tile([128, TQ], BF16, tag="raw",
                                       name="raw")
                        nc.vector.tensor_copy(raw[:], pp[:])
                        sums = ps_sums.tile([65, TQ], F32, tag="sums",
                                            name="sums")
                        nc.tensor.matmul(sums[:], ones2c[:], sq[:],
                                         start=True, stop=True)
                        nc.scalar.activation(rms_sb[:, sl], sums[:],
                                             AF.Sqrt, bias=eps_sb[:],
                                             scale=1.0 / D)
                        nc.vector.reciprocal_approx_fast(
                            out=rrf_sb[:, sl], in_=rms_sb[:, sl])
                        # broadcast 1/rms along partitions via a K=1
                        # outer-product matmul (gpsimd partition ops cost
                        # ~13us on the Q7 cores - far too slow)
                        bb = ps_bb.tile([128, TQ], F32, tag="bb", name="bb")
                        for h in range(HPC):
                            nc.vector.tensor_copy(
                                rec_sb[h][0:1, sl],
                                rrf_sb[64 * h:64 * h + 1, sl])
                            nc.tensor.matmul(
                                bb[:], oneh_sb[0:1, 128 * h:128 * (h + 1)],
                                rec_sb[h][0:1, sl],
                                start=(h == 0), stop=(h == 1))
                        # dst = (raw * gamma[p]) * bb
                        nc.vector.scalar_tensor_tensor(
                            out=dst[:, sl], in0=raw[:], scalar=g_sb[:],
                            in1=bb[:], op0=MUL, op1=MUL)
                # V projection for chunk 0 closes the prologue
                for op in v_proj_ops(0):
                    op()

            # ========== Main loop: attention per chunk, V-proj(c+1) =======
            with (
                tc.tile_pool(name="ep", bufs=6) as ep,
                tc.tile_pool(name="ctxp", bufs=2) as ctxp,
                tc.tile_pool(name="b2p", bufs=2) as b2p,
                tc.tile_pool(name="stgp", bufs=4) as stgp,
                tc.tile_pool(name="ps_tp", bufs=1, space="PSUM") as ps_tp,
                tc.tile_pool(name="ps_st", bufs=2, space="PSUM") as ps_st,
                tc.tile_pool(name="ps_ot", bufs=1, space="PSUM") as ps_ot,
            ):
                for c4 in range(NTQ):
                    sl = slice(TQ * c4, TQ * (c4 + 1))
                    n_tk = 4 * (c4 + 1)
                    vwork = list(v_proj_ops(c4 + 1)) if c4 < 3 else []
                    vw_i = 0
                    ot = [ps_ot.tile([65, TQ], F32, tag=f"ot{h}",
                                     name=f"ot{h}_{c4}")
                          for h in range(HPC)]
                    pairs = [(r, h) for r in range(n_tk)
                             for h in range(HPC)]
                    e_ts = {}
                    # software-pipelined: score(i) issues before pv(i-1) so
                    # the PE never waits on exp; V-proj(c4+1) ops fill the
                    # remaining slack
                    for i, (r, h) in enumerate(pairs):
                        hsl = slice(64 * h, 64 * (h + 1))
                        st = ps_st.tile([128, TQ], F32, tag="st",
                                        name=f"st{h}_{r}")
                        nc.tensor.matmul(
                            st[:], KTn[hsl, 128 * r:128 * (r + 1)],
                            QTn[hsl, sl], start=True, stop=True)
                        e_t = ep.tile([128, TQ], BF16, tag="e",
                                      name=f"e{h}_{r}")
                        nc.scalar.activation(e_t[:], st[:], AF.Exp,
                                             scale=float(D) ** -0.5)
                        diag = r - 4 * c4
                        if diag >= 0:
                            nc.vector.tensor_mul(e_t[:], e_t[:],
                                                 msb[:, diag, :])
                        e_ts[(r, h)] = e_t
                        if i >= 1:
                            rp, hp = pairs[i - 1]
                            nc.tensor.matmul(
                                ot[hp][:],
                                V_sb[:, rp, 65 * hp:65 * hp + 65],
                                e_ts.pop((rp, hp)),
                                start=(rp == 0), stop=(rp == n_tk - 1))
                        if i % 2 == 0 and vw_i < len(vwork):
                            vwork[vw_i]()
                            vw_i += 1
                    rp, hp = pairs[-1]
                    nc.tensor.matmul(
                        ot[hp][:], V_sb[:, rp, 65 * hp:65 * hp + 65],
                        e_ts.pop((rp, hp)),
                        start=(rp == 0), stop=(rp == n_tk - 1))
                    while vw_i < len(vwork):
                        vwork[vw_i]()
                        vw_i += 1
                    # ---- softmax denominators + normalize ----
                    ctxT = ctxp.tile([128, TQ], BF16, tag="ctx",
                                     name=f"ctx{c4}")
                    b2 = ps_mm.tile([128, TQ], F32, tag="mm",
                                    name=f"b2{c4}")
                    for h in range(HPC):
                        nc.vector.tensor_copy(sg[h][0:1, sl],
                                              ot[h][64:65, :])
                        nc.vector.reciprocal_approx_fast(
                            out=sgf[h][0:1, sl], in_=sg[h][0:1, sl])
                        nc.vector.tensor_copy(sgr[h][0:1, sl],
                                              sgf[h][0:1, sl])
                        nc.tensor.matmul(
                            b2[:], oneh_sb[0:1, 128 * h:128 * (h + 1)],
                            sgr[h][0:1, sl], start=(h == 0), stop=(h == 1))
                    # DVE cannot read two PSUM operands; stage b2 in SBUF
                    b2s = b2p.tile([128, TQ], BF16, tag="b2",
                                   name=f"b2s{c4}")
                    nc.vector.tensor_copy(b2s[:], b2[:])
                    for h in range(HPC):
                        nc.vector.scalar_tensor_tensor(
                            out=ctxT[64 * h:64 * (h + 1), :],
                            in0=ot[h][0:64, :],
                            scalar=1.0, in1=b2s[64 * h:64 * (h + 1), :],
                            op0=MUL, op1=MUL)
                    # ---- output projection, DMA per (mu, chunk) ----
                    for mu in range(8):
                        wop = ps_mm.tile([128, TQ], F32, tag="mm",
                                         name=f"wop{mu}_{c4}")
                        nc.tensor.matmul(wop[:],
                                         wo_sb[:, 128 * mu:128 * (mu + 1)],
                                         ctxT[:], start=True, stop=True)
                        stg = stgp.tile([128, TQ], BF16, tag="stg",
                                        name=f"stg{mu}_{c4}")
                        if mu % 4 == 1:
                            nc.scalar.activation(stg[:], wop[:], AF.Copy)
                        else:
                            nc.vector.tensor_copy(stg[:], wop[:])
                        nc.sync.dma_start(out=outT_d[:, mu, sl],
                                          in_=stg[:])

    nc.compile()
    return nc


_NC_CACHE = None


def _get_nc():
    global _NC_CACHE
    if _NC_CACHE is None:
        _NC_CACHE = build_nc()
    return _NC_CACHE


def _make_in_maps(x, w_q, w_k, w_v, w_o, q_gamma, k_gamma):
    x = np.asarray(x, dtype=np.float32).reshape(T, C)
    # xT[p, c, t] = x[t, c*128+p]
    xT = np.ascontiguousarray(
        x.reshape(T, 8, 128).transpose(2, 1, 0)).astype(NP_BF16)

    p = np.arange(128)
    f = np.arange(TQ)
    masks = np.zeros((128, 4, TQ), dtype=NP_BF16)
    for s in range(4):
        masks[:, s, :] = (f[None, :] >= (p[:, None] + 128 * s)).astype(
            NP_BF16)

    gq = np.tile(np.asarray(q_gamma, np.float32), 2).reshape(128, 1)
    gk = np.tile(np.asarray(k_gamma, np.float32), 2).reshape(128, 1)
    onescol = np.zeros((128, 65), dtype=NP_BF16)
    onescol[0:64, 0] = 1
    onescol[64:128, 64] = 1
    oneh = np.zeros((1, 256), dtype=NP_BF16)
    oneh[0, 0:64] = 1          # head0 block in first 128 cols
    oneh[0, 192:256] = 1       # head1 block in second 128 cols
    ident = np.eye(128, dtype=NP_BF16)

    common = dict(xT=xT, masks=masks, gq=gq, gk=gk, onescol=onescol,
                  oneh=oneh, ident=ident)

    in_maps = []
    for i in range(NCORES):
        rows = slice(JPC * i, JPC * (i + 1))

        def wsw(w):
            # [p, c, j] = W[rows][j, c*128+p]
            W = np.asarray(w, np.float32)[rows]           # [128, C]
            return np.ascontiguousarray(
                W.reshape(128, 8, 128).transpose(2, 1, 0)).astype(NP_BF16)

        wo = np.asarray(w_o, np.float32)[:, rows].T        # [128, C]
        in_maps.append(dict(common, wq=wsw(w_q), wk=wsw(w_k), wv=wsw(w_v),
                            wo=np.ascontiguousarray(wo).astype(NP_BF16)))
    return in_maps


def _run(x, w_q, w_k, w_v, w_o, q_gamma, k_gamma, trace=False):
    import time

    from concourse.bass_utils import run_bass_kernel_spmd
    nc = _get_nc()
    in_maps = _make_in_maps(x, w_q, w_k, w_v, w_o, q_gamma, k_gamma)
    res = None
    for attempt in range(3):
        try:
            res = run_bass_kernel_spmd(nc, in_maps, list(range(NCORES)),
                                       trace=trace)
            break
        except Exception:
            # rare transient NRT_EXEC_UNIT_UNRECOVERABLE under axon; the
            # terminal resets the device on the next load
            if attempt == 2:
                raise
            time.sleep(3.0)
    acc = np.zeros((128, 8, T), dtype=np.float64)
    for r in res.results:
        acc += r["outT"].astype(np.float64)
    # out[t, m*128+p] = acc[p, m, t]
    out = acc.transpose(2, 1, 0).reshape(T, C).astype(np.float32)
    return out.reshape(1, T, C), res


def kernel(x, w_q, w_k, w_v, w_o, q_gamma, k_gamma):
    out, _ = _run(x, w_q, w_k, w_v, w_o, q_gamma, k_gamma, trace=False)
    return out
